# revision 1
# baseline (speedup 1.0000x reference)
"""Trainium2 Bass kernel: LayerNorm + multi-head self-attention + residual.

Computes, per batch b:
    xn = LayerNorm(x[b]) * g + b
    q/k/v = xn @ W{q,k,v}.T + b{q,k,v}      (16 heads, dh=64)
    attn  = softmax(q k^T + maskbias, over keys)
    out   = x + (attn @ (v*mask)) reshaped

Sharding over 8 cores: batch (2-way) x head-group (4-way, 4 heads each).
Each core gets full x[b] (for LayerNorm) plus its 256-column slice of the
Q/K/V weights, and produces a [2048, 256] slice of the output.

Host-side folding: LN's g is folded into the weight columns and LN's b into
the projection biases (Q = (x-mu)*rstd @ (W*g).T + (W@b + bq)), so the device
only computes the standardized activation xc = (x - mu) * rstd.

Precision: all matmul operands fp16 (full PE rate; validated ~5e-3 normalized
absmax error vs the fp32 reference), softmax weights bf16 (needs fp32-range
exponent), all accumulation fp32 in PSUM.

On-device dataflow (per core):
  1. LN stats in [n, d] layout (bn_stats/bn_aggr), affine -> fp16, PE
     transposes of 128x128 blocks into xnT [d, n]; the PSUM->SBUF copies run
     on ScalarE, which is otherwise idle during this phase.
  2. Projections: Q^T,K^T as [c, n] (c = head dims stacked in pairs of
     heads), V as [m, c] with bias via a rank-1 ones x bias matmul; V stored
     with a ones column per head (65 cols) so the AV matmul also produces
     softmax denominators (row 64 of Y^T).
  3. Attention per head-pair, n in slices of 1024 (2-bank psum tiles so each
     EXP instruction covers 1024 elements, amortizing ACT's ~352-cycle
     per-instruction overhead): S^T[m-chunk, n] = K^T.T @ Q^T (K=64
     contraction; the two heads sit in row groups 0-63/64-127);
     P = exp(S^T + maskbias[m]) on ScalarE (per-partition bias AP; no
     max-subtraction needed since |S| <~ 50 fits fp32/bf16 range);
     Y^T[65, 512] += V'[m,65].T @ P-half, accumulated over m-chunks.
     Consecutive matmuls share lhsT where possible.
  4. PE-transpose Y^T [65,128] blocks -> [n, 65]; multiply by 1/denom
     (column 64, now a per-partition scalar), add residual x, DMA out.

Measured on HW: 305.8 us for the full 8-core launch (slowest core, NTFF),
normalized absmax relative error 4.97e-3.
"""

import sys

for _p in ("/opt/trn_rl_repo",):
    if _p not in sys.path:
        sys.path.insert(0, _p)

import numpy as np

import concourse.bacc as bacc
import concourse.bass as bass
import concourse.mybir as mybir
import concourse.tile as tile
from concourse.masks import make_identity

F32 = mybir.dt.float32
F16 = mybir.dt.float16
BF16 = mybir.dt.bfloat16

T = 2048          # sequence length
D = 1024          # model dim
HC = 4            # heads per core
DH = 64           # head dim
CC = HC * DH      # columns per core (256)
NC = T // 128     # 16 n/m chunks of 128
DC = D // 128     # 8 d chunks

_CACHE = {}


def _maybe_patch_ldw_opt():
    """Optionally re-enable walrus's redundant-LDWEIGHTS elimination.

    concourse hardcodes --enable-ldw-opt=false; our matmul streams reuse the
    stationary operand across consecutive matmuls, so the redundant weight
    loads are pure overhead. Gated by env until validated.
    """
    import os
    if os.environ.get("KERNEL_LDW_OPT") != "1" or _CACHE.get("ldw_patched"):
        return
    from concourse import bass_utils as _bu
    _orig = _bu.run_command

    def _run(argv, **kw):
        argv = ["--enable-ldw-opt=true" if a == "--enable-ldw-opt=false" else a
                for a in argv]
        return _orig(argv, **kw)

    _bu.run_command = _run
    _CACHE["ldw_patched"] = True


def build_bass():
    # Bacc (not plain Bass): its finalize() runs generate_event_semaphores,
    # which splits multi-waits into EventSemaphore instructions — walrus
    # rejects >1 sync wait on most engine instruction structs.
    nc = bacc.Bacc()

    x_d = nc.declare_dram_parameter("x", [T, D], F32, isOutput=False)
    xres_d = nc.declare_dram_parameter("xres", [T, CC], F32, isOutput=False)
    wqt_d = nc.declare_dram_parameter("wqt", [D, CC], F16, isOutput=False)
    wkt_d = nc.declare_dram_parameter("wkt", [D, CC], F16, isOutput=False)
    wvt_d = nc.declare_dram_parameter("wvt", [D, CC], F16, isOutput=False)
    bq_d = nc.declare_dram_parameter("bq2", [128, 2], F32, isOutput=False)
    bk_d = nc.declare_dram_parameter("bk2", [128, 2], F32, isOutput=False)
    bvr_d = nc.declare_dram_parameter("bvr", [1, CC], F16, isOutput=False)
    mb_d = nc.declare_dram_parameter("mbias", [128, NC], F32, isOutput=False)
    mm_d = nc.declare_dram_parameter("mmul", [128, NC], F32, isOutput=False)
    out_d = nc.declare_dram_parameter("out", [T, CC], F32, isOutput=True)

    with tile.TileContext(nc) as tc:
        _body(tc, x_d, xres_d, wqt_d, wkt_d, wvt_d,
              bq_d, bk_d, bvr_d, mb_d, mm_d, out_d)
    nc.finalize()
    return nc


def _body(tc, x_d, xres_d, wqt_d, wkt_d, wvt_d,
          bq_d, bk_d, bvr_d, mb_d, mm_d, out_d):
    nc = tc.nc
    import contextlib
    ctx = contextlib.ExitStack()
    with ctx:
        consts = ctx.enter_context(tc.tile_pool(name="consts", bufs=1))
        persist = ctx.enter_context(tc.tile_pool(name="persist", bufs=1))
        xcpool = ctx.enter_context(tc.tile_pool(name="xcpool", bufs=3))
        stats = ctx.enter_context(tc.tile_pool(name="stats", bufs=4))
        ppool = ctx.enter_context(tc.tile_pool(name="ppool", bufs=6))
        ytpool = ctx.enter_context(tc.tile_pool(name="ytpool", bufs=4))
        recpool = ctx.enter_context(tc.tile_pool(name="recpool", bufs=4))
        outpool = ctx.enter_context(tc.tile_pool(name="outpool", bufs=3))
        scpsum = ctx.enter_context(tc.tile_pool(name="scpsum", bufs=2, space="PSUM"))
        avpsum = ctx.enter_context(tc.tile_pool(name="avpsum", bufs=4, space="PSUM"))

        # ---- constants -------------------------------------------------
        wq_sb = consts.tile([128, DC, CC], F16)
        wk_sb = consts.tile([128, DC, CC], F16)
        wv_sb = consts.tile([128, DC, CC], F16)
        nc.sync.dma_start(wq_sb, wqt_d[:].rearrange("(o p) c -> p o c", p=128))
        nc.sync.dma_start(wk_sb, wkt_d[:].rearrange("(o p) c -> p o c", p=128))
        nc.sync.dma_start(wv_sb, wvt_d[:].rearrange("(o p) c -> p o c", p=128))
        bq_t = consts.tile([128, 2], F32)
        bk_t = consts.tile([128, 2], F32)
        nc.sync.dma_start(bq_t, bq_d[:])
        nc.sync.dma_start(bk_t, bk_d[:])
        bvr_t = consts.tile([1, CC], F16)
        nc.sync.dma_start(bvr_t, bvr_d[:])
        mb_t = consts.tile([128, NC], F32)
        mm_t = consts.tile([128, NC], F32)
        nc.sync.dma_start(mb_t, mb_d[:])
        nc.sync.dma_start(mm_t, mm_d[:])

        # absorb const-DMA completion waits on the engines that later read
        # these tiles via scalar-pointer operands (those instruction structs
        # can encode only one sync wait)
        touch_v = consts.tile([128, 1], F32)
        nc.vector.tensor_copy(touch_v, bq_t[:, 0:1])
        nc.vector.tensor_copy(touch_v, bk_t[:, 0:1])
        nc.vector.tensor_copy(touch_v, mm_t[:, 0:1])
        touch_a = consts.tile([128, 1], F32)
        nc.scalar.copy(touch_a, mb_t[:, 0:1])

        ident32 = consts.tile([128, 128], F32)
        make_identity(nc, ident32)
        ident16 = consts.tile([128, 128], F16)
        make_identity(nc, ident16)
        ones1 = consts.tile([1, 128], F16)
        nc.vector.memset(ones1, 1.0)
        eps_t = consts.tile([128, 1], F32)
        nc.vector.memset(eps_t, 1e-5)

        # ---- persistent activations -----------------------------------
        # x and xres are loaded once into persistent tiles (streamed pool
        # slots would give the reload DMAs >2 sync waits).
        x_all = persist.tile([128, NC, D], F32)
        xv = x_d[:].rearrange("(o p) d -> p o d", p=128)
        for q in range(4):
            nc.sync.dma_start(x_all[:, 4 * q:4 * (q + 1), :],
                              xv[:, 4 * q:4 * (q + 1), :])
        xres_all = persist.tile([128, NC, CC], F32)
        nc.sync.dma_start(xres_all,
                          xres_d[:].rearrange("(o p) c -> p o c", p=128))
        xnT = persist.tile([128, DC, T], F16)       # xn^T (g,b folded on host)
        qT = persist.tile([128, 2, T], F16)         # Q^T per head-pair
        kT = persist.tile([128, 2, T], F16)
        vP = persist.tile([128, NC, HC * (DH + 1)], BF16)  # V' with ones cols

        # ones columns of V' (softmax denominator trick)
        vP4 = vP[:].rearrange("p i (h c) -> p i h c", c=DH + 1)
        nc.vector.memset(vP4[:, :, :, DH], 1.0)

        # ---- phase 1: LayerNorm + DMA transpose ------------------------
        for ic in range(NC):
            x_t = x_all[:, ic, :]
            st = stats.tile([128, 2, 6], F32, tag="st")
            nc.vector.bn_stats(st[:, 0, :], x_t[:, 0:512])
            nc.vector.bn_stats(st[:, 1, :], x_t[:, 512:1024])
            mv = stats.tile([128, 2], F32, tag="mv")
            nc.vector.bn_aggr(mv, st)
            rstd = stats.tile([128, 1], F32, tag="rstd")
            nc.scalar.activation(rstd, mv[:, 1:2],
                                 mybir.ActivationFunctionType.Sqrt,
                                 bias=eps_t, scale=1.0)
            nc.vector.reciprocal(rstd, rstd)
            xc = xcpool.tile([128, D], F16, tag="xc")
            nc.vector.tensor_scalar(
                out=xc, in0=x_t, scalar1=mv[:, 0:1], scalar2=rstd,
                op0=mybir.AluOpType.subtract, op1=mybir.AluOpType.mult)
            for dc in range(DC):
                tps = avpsum.tile([128, 512], F32, tag="av", name="tps").bitcast(F16)[:, 0:128]
                nc.tensor.transpose(tps, xc[:, 128 * dc:128 * (dc + 1)],
                                    ident16)
                # copy PSUM->SBUF on ACT: it is idle during LN and DVE is
                # the LN-phase bottleneck (stats + affine)
                dst = xnT[:, dc, 128 * ic:128 * (ic + 1)]
                if dc >= 6:
                    nc.vector.tensor_copy(out=dst, in_=tps)
                else:
                    nc.scalar.copy(dst, tps)

        # ---- phase 2a: V projection (+bias, *mask, bf16) ---------------
        def v_proj():
            for ic in range(NC):
              psv = avpsum.tile([128, 512], F32, tag="av", name="psv")[:, 0:256]
              for dc in range(DC):
                  nc.tensor.matmul(psv,
                                   lhsT=xnT[:, dc, 128 * ic:128 * (ic + 1)],
                                   rhs=wv_sb[:, dc, :],
                                   start=(dc == 0), stop=False)
              # rank-1 bias add: ones[1,128].T @ bv[1,CC]
              nc.tensor.matmul(psv, lhsT=ones1, rhs=bvr_t,
                               start=False, stop=True)
              for h in range(HC):
                  nc.vector.tensor_scalar_mul(
                      out=vP[:, ic, (DH + 1) * h:(DH + 1) * h + DH],
                      in0=psv[:, DH * h:DH * (h + 1)],
                      scalar1=mm_t[:, ic:ic + 1])

        def qk_proj(pg):
            # dc-outer with two n-slices of 1024 alive: each weight chunk is
            # the stationary operand for 2 consecutive matmuls.
            for jp in range(2):
                for w_sb, dstT, b_t in ((wk_sb, kT, bk_t), (wq_sb, qT, bq_t)):
                    ps0 = avpsum.tile([128, 512], F32, tag="av", name="pj0")
                    ps1 = avpsum.tile([128, 512], F32, tag="av", name="pj1")
                    for dc in range(DC):
                        lhsT = w_sb[:, dc, 128 * pg:128 * (pg + 1)]
                        nc.tensor.matmul(
                            ps0, lhsT=lhsT,
                            rhs=xnT[:, dc, 1024 * jp:1024 * jp + 512],
                            start=(dc == 0), stop=(dc == DC - 1))
                        nc.tensor.matmul(
                            ps1, lhsT=lhsT,
                            rhs=xnT[:, dc, 1024 * jp + 512:1024 * (jp + 1)],
                            start=(dc == 0), stop=(dc == DC - 1))
                    nc.vector.tensor_scalar_add(
                        out=dstT[:, pg, 1024 * jp:1024 * jp + 512], in0=ps0,
                        scalar1=b_t[:, pg:pg + 1])
                    nc.vector.tensor_scalar_add(
                        out=dstT[:, pg, 1024 * jp + 512:1024 * (jp + 1)],
                        in0=ps1, scalar1=b_t[:, pg:pg + 1])

        def attention(pg):
            hA, hB = 2 * pg, 2 * pg + 1
            for j2 in range(2):          # n-slices of 1024
                nsl = slice(1024 * j2, 1024 * (j2 + 1))
                n0 = 1024 * j2
                yA = [avpsum.tile([128, 512], F32, tag="av",
                                  name=f"yA{half}")[0:DH + 1]
                      for half in range(2)]
                yB = [avpsum.tile([128, 512], F32, tag="av",
                                  name=f"yB{half}")[0:DH + 1]
                      for half in range(2)]
                for ic in range(NC):
                    msl = slice(128 * ic, 128 * (ic + 1))
                    scA = scpsum.tile([128, 1024], F32, tag="sc", name="scA")
                    scB = scpsum.tile([128, 1024], F32, tag="sc", name="scB")
                    # A then B: row groups 0-63 / 64-127 -> can overlap in PE
                    kA = kT[0:DH, pg, msl]
                    kB = kT[DH:128, pg, msl]
                    # alternate row groups (A: 0-63, B: 64-127) so adjacent
                    # matmuls can execute concurrently in the PE array
                    nc.tensor.matmul(scA[:, 0:512], lhsT=kA,
                                     rhs=qT[0:DH, pg, n0:n0 + 512],
                                     start=True, stop=True)
                    nc.tensor.matmul(scB[:, 0:512], lhsT=kB,
                                     rhs=qT[DH:128, pg, n0:n0 + 512],
                                     start=True, stop=True)
                    nc.tensor.matmul(scA[:, 512:1024], lhsT=kA,
                                     rhs=qT[0:DH, pg, n0 + 512:n0 + 1024],
                                     start=True, stop=True)
                    nc.tensor.matmul(scB[:, 512:1024], lhsT=kB,
                                     rhs=qT[DH:128, pg, n0 + 512:n0 + 1024],
                                     start=True, stop=True)
                    pA = ppool.tile([128, 1024], BF16, tag="p")
                    nc.scalar.activation(pA, scA,
                                         mybir.ActivationFunctionType.Exp,
                                         bias=mb_t[:, ic:ic + 1], scale=1.0)
                    pB = ppool.tile([128, 1024], BF16, tag="p")
                    nc.scalar.activation(pB, scB,
                                         mybir.ActivationFunctionType.Exp,
                                         bias=mb_t[:, ic:ic + 1], scale=1.0)
                    vA = vP[:, ic, (DH + 1) * hA:(DH + 1) * (hA + 1)]
                    vB = vP[:, ic, (DH + 1) * hB:(DH + 1) * (hB + 1)]
                    nc.tensor.matmul(yA[0], lhsT=vA, rhs=pA[:, 0:512],
                                     start=(ic == 0), stop=(ic == NC - 1))
                    nc.tensor.matmul(yA[1], lhsT=vA, rhs=pA[:, 512:1024],
                                     start=(ic == 0), stop=(ic == NC - 1))
                    nc.tensor.matmul(yB[0], lhsT=vB, rhs=pB[:, 0:512],
                                     start=(ic == 0), stop=(ic == NC - 1))
                    nc.tensor.matmul(yB[1], lhsT=vB, rhs=pB[:, 512:1024],
                                     start=(ic == 0), stop=(ic == NC - 1))

                # normalize + residual + store
                for half in range(2):
                    ytA = ytpool.tile([DH + 1, 512], F32, tag="yt")
                    nc.vector.tensor_copy(ytA, yA[half])
                    ytB = ytpool.tile([DH + 1, 512], F32, tag="yt")
                    nc.scalar.copy(ytB, yB[half])
                    for k in range(4):
                        ic_g = 8 * j2 + 4 * half + k
                        rows = slice(128 * ic_g, 128 * (ic_g + 1))
                        ksl = slice(128 * k, 128 * (k + 1))
                        out_t = outpool.tile([128, 128], F32, tag="out")
                        for hh, yt in ((0, ytA), (1, ytB)):
                            otp = avpsum.tile([128, 512], F32, tag="av", name="otp")[:, 0:DH + 1]
                            nc.tensor.transpose(otp, yt[:, ksl],
                                                ident32[0:DH + 1, 0:DH + 1])
                            rec = recpool.tile([128, 1], F32, tag="rec")
                            nc.vector.reciprocal(rec, otp[:, DH:DH + 1])
                            nc.vector.tensor_scalar_mul(
                                out=out_t[:, DH * hh:DH * (hh + 1)],
                                in0=otp[:, 0:DH], scalar1=rec)
                        nc.vector.tensor_add(
                            out_t, out_t,
                            xres_all[:, ic_g, 128 * pg:128 * (pg + 1)])
                        nc.sync.dma_start(
                            out_d[rows, 128 * pg:128 * (pg + 1)], out_t)

        qk_proj(0)
        v_proj()
        attention(0)
        qk_proj(1)
        attention(1)


def _host_in_map(core, x, src_mask, ln_g, ln_b, Wq, bq, Wk, bk, Wv, bv):
    b, hg = divmod(core, 4)
    cs = CC * hg
    xb = np.ascontiguousarray(x[b], dtype=np.float32)
    mask = np.asarray(src_mask[b, :, 0], dtype=np.float32)
    ln_g = np.asarray(ln_g, np.float32)
    ln_b = np.asarray(ln_b, np.float32)

    def wfold(W):
        # fold LN scale g into weight columns: (W * g).T, fp16
        Ws = np.asarray(W, np.float32)[cs:cs + CC, :]
        return np.ascontiguousarray((Ws * ln_g[None, :]).T).astype(np.float16)

    def bfold(W, bb):
        # fold LN shift b into the projection bias: W @ b + bias
        Ws = np.asarray(W, np.float32)[cs:cs + CC, :]
        return Ws @ ln_b + np.asarray(bb, np.float32)[cs:cs + CC]

    return {
        "x": xb,
        "xres": np.ascontiguousarray(xb[:, cs:cs + CC]),
        "wqt": wfold(Wq),
        "wkt": wfold(Wk),
        "wvt": wfold(Wv),
        "bq2": np.ascontiguousarray(bfold(Wq, bq).reshape(2, 128).T),
        "bk2": np.ascontiguousarray(bfold(Wk, bk).reshape(2, 128).T),
        "bvr": bfold(Wv, bv).reshape(1, CC).astype(np.float16),
        "mbias": np.ascontiguousarray(
            ((1.0 - mask) * -1000000.0).reshape(NC, 128).T),
        "mmul": np.ascontiguousarray(mask.reshape(NC, 128).T),
    }


def kernel(x, src_mask, ln_g, ln_b, Wq, bq, Wk, bk, Wv, bv, _trace=False,
           _tmpdir=None):
    x = np.asarray(x, dtype=np.float32)
    B = x.shape[0]
    _maybe_patch_ldw_opt()
    if "nc" not in _CACHE:
        _CACHE["nc"] = build_bass()
    nc = _CACHE["nc"]

    from concourse.bass_utils import run_bass_kernel_spmd
    in_maps = [
        _host_in_map(c, x, np.asarray(src_mask), np.asarray(ln_g),
                     np.asarray(ln_b), np.asarray(Wq), np.asarray(bq),
                     np.asarray(Wk), np.asarray(bk), np.asarray(Wv),
                     np.asarray(bv))
        for c in range(8)
    ]
    res = run_bass_kernel_spmd(nc, in_maps, core_ids=list(range(8)),
                               trace=_trace, tmpdir=_tmpdir)
    out = np.empty((B, T, D), dtype=np.float32)
    for c in range(8):
        b, hg = divmod(c, 4)
        out[b, :, CC * hg:CC * (hg + 1)] = res.results[c]["out"]
    if _trace:
        _CACHE["last_result"] = res
    return out

